# revision 7
# baseline (speedup 1.0000x reference)
"""Trainium2 Bass kernel for nn_AdvantageNetwork (gnn_message_passing).

Math (per batch b of B=4096, N=32 agents, d_in=256, D=256):
  x = concat(states, actions)                       [B,N,256]
  K = x Wk^T ; Q = x Wq^T ; V = x Wv^T              [B,N,256]
  score[b,i,j] = <Q[b,j], K[b,i]> / 16
  w = softmax_j(score)                              [B,N,N]
  weighted = w V / N                                [B,N,256]
  h = leaky_relu(weighted W1^T, 0.01)               [B,N,64]
  adv = h W2^T                                      [B,N,16]
  returns (adv, w[..., None])

Kernel algebra:
  score = x A x^T / 16 with A = Wk^T Wq     (skips separate K,Q projections)
  V' = x (W1 Wv)^T / 32                     (fuses V proj, W1 matmul and /N)
  leaky_relu(y) = relu(y) - 0.01 relu(-y)   (both Relus live in the exp ACT
      table set -> no act-table reloads; subtraction folds into the adv
      matmul as a second accumulate against -W2^T)

Sharding: data-parallel over batch across 8 NeuronCores (512 batches/core).
"""
import numpy as np
from contextlib import ExitStack

import concourse.bass as bass
import concourse.tile as tile
from concourse import bacc, mybir
from concourse.masks import make_identity

N_CORES = 8
B, N, OBS, ACT = 4096, 32, 240, 16
D = 256
BS = B // N_CORES            # 512 batches per core
M = BS * N                   # 16384 rows per core
MT = 512                     # rows per m-tile
NMT = M // MT                # 32 m-tiles
f32 = mybir.dt.float32
f32r = mybir.dt.float32r
AF = mybir.ActivationFunctionType


def build_kernel(reps: int = 1):
    nc = bacc.Bacc("TRN2", target_bir_lowering=False, debug=False,
                   num_devices=N_CORES)
    xin = nc.dram_tensor("xin", [M, D], f32, kind="ExternalInput").ap()
    Wk = nc.dram_tensor("Wk", [D, D], f32, kind="ExternalInput").ap()
    Wq = nc.dram_tensor("Wq", [D, D], f32, kind="ExternalInput").ap()
    Wv = nc.dram_tensor("Wv", [D, D], f32, kind="ExternalInput").ap()
    W1 = nc.dram_tensor("W1", [64, D], f32, kind="ExternalInput").ap()
    W2 = nc.dram_tensor("W2", [16, 64], f32, kind="ExternalInput").ap()
    # fused output: cols 0:32 = softmax w, cols 32:48 = adv
    wadv = nc.dram_tensor("wadv", [M, 48], f32, kind="ExternalOutput").ap()

    with tile.TileContext(nc) as tc, ExitStack() as ctx:
        consts = ctx.enter_context(tc.tile_pool(name="consts", bufs=1))

        ident = consts.tile([128, 128], f32)
        make_identity(nc, ident)

        # ---------------- weight setup (once) ----------------
        A_sb = consts.tile([128, 2, 256], f32r)   # A = Wk^T Wq, din on part
        Wv1T = consts.tile([128, 2, 64], f32r)    # (W1 Wv)^T / 32, din on part
        W2T = consts.tile([64, 16], f32)          # W2^T
        W2Tn = consts.tile([64, 16], f32)         # -W2^T

        with tc.tile_pool(name="setup_sb", bufs=1) as ssb, \
             tc.tile_pool(name="setup_ps", bufs=1, space="PSUM") as sps:
            wk_sb = ssb.tile([128, 2, 256], f32)   # [k part, k blk, din]
            wq_sb = ssb.tile([128, 2, 256], f32)
            wv_sb = ssb.tile([128, 2, 256], f32)
            w1_sb = ssb.tile([64, 256], f32)
            w2_sb = ssb.tile([16, 64], f32)
            nc.sync.dma_start(out=wk_sb, in_=Wk.rearrange("(kb k) d -> k kb d", k=128))
            nc.sync.dma_start(out=wq_sb, in_=Wq.rearrange("(kb k) d -> k kb d", k=128))
            nc.sync.dma_start(out=wv_sb, in_=Wv.rearrange("(kb k) d -> k kb d", k=128))
            nc.sync.dma_start(out=w1_sb, in_=W1)
            nc.sync.dma_start(out=w2_sb, in_=W2)

            # A rows block db: psum = sum_kb Wk[:,kb,db-slice].T @ Wq[:,kb,:]
            for db in range(2):
                a_ps = sps.tile([128, 256], f32, tag="a")
                for kb in range(2):
                    nc.tensor.matmul(a_ps, wk_sb[:, kb, 128 * db:128 * db + 128],
                                     wq_sb[:, kb, :],
                                     start=(kb == 0), stop=(kb == 1))
                nc.scalar.copy(A_sb[:, db, :], a_ps)

            # W1T [128, 2(kb), 64] via PE transpose of W1 [64, 256]
            w1t = ssb.tile([128, 2, 64], f32)
            for kb in range(2):
                t_ps = sps.tile([128, 64], f32, tag="t")
                nc.tensor.transpose(t_ps, w1_sb[:, 128 * kb:128 * kb + 128],
                                    ident[0:64, 0:64])
                nc.scalar.copy(w1t[:, kb, :], t_ps)

            # Wv1T din-block db = sum_kb Wv[:,kb,db-slice].T @ W1T[:,kb,:], / N
            for db in range(2):
                v_ps = sps.tile([128, 64], f32, tag="t")
                for kb in range(2):
                    nc.tensor.matmul(v_ps, wv_sb[:, kb, 128 * db:128 * db + 128],
                                     w1t[:, kb, :],
                                     start=(kb == 0), stop=(kb == 1))
                nc.scalar.mul(Wv1T[:, db, :], v_ps, 1.0 / N)

            # W2T = W2^T, W2Tn = -W2^T
            t2_ps = sps.tile([64, 16], f32, tag="t")
            nc.tensor.transpose(t2_ps, w2_sb, ident[0:16, 0:16])
            nc.scalar.copy(W2T, t2_ps)
            nc.scalar.mul(W2Tn, t2_ps, -1.0)

        # block-diag w^T holders [128(b,j), 4(sub), 128(b,i)]:
        # zeroed once, diagonals rewritten per m-tile; 2 rotating buffers
        wblk0 = consts.tile([128, 4, 128], f32)
        wblk1 = consts.tile([128, 4, 128], f32)
        nc.vector.memset(wblk0, 0.0)
        nc.vector.memset(wblk1, 0.0)
        wblks = [wblk0, wblk1]

        # ---------------- pools ----------------
        x_p = ctx.enter_context(tc.tile_pool(name="x", bufs=2))
        xT_p = ctx.enter_context(tc.tile_pool(name="xT", bufs=2))
        zT_p = ctx.enter_context(tc.tile_pool(name="zT", bufs=2))
        vp_p = ctx.enter_context(tc.tile_pool(name="vp", bufs=6))
        wt_p = ctx.enter_context(tc.tile_pool(name="wt", bufs=2))
        out_p = ctx.enter_context(tc.tile_pool(name="out", bufs=2))
        small_p = ctx.enter_context(tc.tile_pool(name="small", bufs=3))
        xt_ps_p = ctx.enter_context(tc.tile_pool(name="xt_ps", bufs=2, space="PSUM"))
        zt_ps_p = ctx.enter_context(tc.tile_pool(name="zt_ps", bufs=2, space="PSUM"))
        vp_ps_p = ctx.enter_context(tc.tile_pool(name="vp_ps", bufs=2, space="PSUM"))
        sub_ps_p = ctx.enter_context(tc.tile_pool(name="sub_ps", bufs=2, space="PSUM"))

        for rep in range(reps):
            for mt in range(NMT):
                r0 = mt * MT
                # ---- load x: one DMA, [128, 4(sub), 256]
                x_all = x_p.tile([128, 4, D], f32, tag="x")
                nc.sync.dma_start(
                    out=x_all,
                    in_=xin[r0:r0 + MT, :].rearrange("(s p) d -> p s d", p=128))

                # ---- transpose x -> xT [128(din), 2(din blk), 512(m)] f32r
                xT = xT_p.tile([128, 2, MT], f32r, tag="xT")
                for s in range(4):
                    tp = xt_ps_p.tile([128, 2, 128], f32, tag="xtp")
                    for db in range(2):
                        nc.tensor.transpose(tp[:, db, :],
                                            x_all[:, s, 128 * db:128 * db + 128],
                                            ident)
                    nc.scalar.copy(xT[:, :, 128 * s:128 * s + 128], tp)

                # ---- zT [128(d'), 2(d' blk), 512(m)] f32 : z = x A
                zT = zT_p.tile([128, 2, MT], f32, tag="zT")
                for qb in range(2):
                    z_ps = zt_ps_p.tile([128, MT], f32, tag="zps")
                    for db in range(2):
                        nc.tensor.matmul(z_ps, A_sb[:, db, 128 * qb:128 * qb + 128],
                                         xT[:, db, :],
                                         start=(db == 0), stop=(db == 1))
                    nc.vector.tensor_copy(zT[:, qb, :], z_ps)

                wblk = wblks[mt % 2]
                wt_all = wt_p.tile([32, 4, 128], f32, tag="wt")
                outst = out_p.tile([128, 4, 48], f32, tag="outst")
                vp_list = []

                # ---- per 128-row subtile (4 batches each)
                for s in range(4):
                    ms = 128 * s
                    # V' [128(m), 64] f32 = x Wv1^T / N
                    vps = vp_ps_p.tile([128, 64], f32, tag="vps")
                    for db in range(2):
                        nc.tensor.matmul(vps, xT[:, db, ms:ms + 128],
                                         Wv1T[:, db, :],
                                         start=(db == 0), stop=(db == 1))
                    vp = vp_p.tile([128, 64], f32, tag="vp")
                    nc.vector.tensor_copy(vp, vps)
                    vp_list.append(vp)

                    # score psum [128(4b x i), 32(j)]: col-tiled per batch
                    sub = sub_ps_p.tile([128, 512], f32, tag="sub")
                    sc = sub[:, 0:32]
                    for b4 in range(4):
                        c0 = ms + 32 * b4
                        for db in range(2):
                            nc.tensor.matmul(sc[32 * b4:32 * b4 + 32, :],
                                             zT[:, db, c0:c0 + 32],
                                             xT[:, db, c0:c0 + 32].bitcast(f32),
                                             start=(db == 0), stop=(db == 1),
                                             tile_position=(0, 32 * b4))

                    # softmax over j (free axis): no max-subtract (|score|<~4)
                    E = small_p.tile([128, 32], f32, tag="E")
                    S = small_p.tile([128, 1], f32, tag="S")
                    nc.scalar.activation(out=E, in_=sc, func=AF.Exp,
                                         scale=1.0 / 16.0, accum_out=S)
                    R = small_p.tile([128, 1], f32, tag="R")
                    nc.vector.reciprocal(R, S)
                    wsb = outst[:, s, 0:32]
                    nc.vector.tensor_scalar_mul(wsb, E, R)

                    # w^T into stacked holder (psum) then sbuf
                    wt_ps = sub[0:32, 32:160]
                    nc.tensor.transpose(wt_ps, wsb, ident)
                    nc.scalar.copy(wt_all[:, s, :], wt_ps)

                # ---- block-diag build: 4 DMAs move all 4 subtiles at once
                for b4 in range(4):
                    sl = slice(32 * b4, 32 * b4 + 32)
                    nc.sync.dma_start(out=wblk[sl, :, sl], in_=wt_all[:, :, sl])

                for s in range(4):
                    sub2 = sub_ps_p.tile([128, 512], f32, tag="sub")
                    # weightedT [64(h), 128(4b x i)] = V'^T @ wblk (incl /N)
                    wtd_ps = sub2[0:64, 0:128]
                    nc.tensor.matmul(wtd_ps, vp_list[s], wblk[:, s, :],
                                     start=True, stop=True)

                    # leaky_relu = relu(y) - 0.01 relu(-y), split ACT/DVE
                    hTp = small_p.tile([64, 128], f32, tag="hTp")
                    nc.scalar.activation(out=hTp, in_=wtd_ps, func=AF.Relu)
                    hTn = small_p.tile([64, 128], f32, tag="hTn")
                    nc.vector.tensor_scalar(out=hTn, in0=wtd_ps, scalar1=-0.01,
                                            scalar2=0.0,
                                            op0=mybir.AluOpType.mult,
                                            op1=mybir.AluOpType.max)

                    # adv [128(4b x i), 16] = hTp^T W2T + hTn^T (-W2T)... note
                    # leaky = hTp - 0.01 relu(-y) and hTn = relu(-0.01 y) ≥ 0
                    adv_ps = sub2[:, 128:144]
                    nc.tensor.matmul(adv_ps, hTp, W2T, start=True, stop=False)
                    nc.tensor.matmul(adv_ps, hTn, W2Tn, start=False, stop=True)
                    nc.vector.tensor_copy(outst[:, s, 32:48], adv_ps)

                # ---- one fused output DMA per m-tile
                nc.sync.dma_start(
                    out=wadv[r0:r0 + MT, :].rearrange("(s p) d -> p s d", p=128),
                    in_=outst)

    nc.compile()
    return nc


_CACHE = {}


def _get_built(reps: int = 1):
    if reps not in _CACHE:
        _CACHE[reps] = build_kernel(reps)
    return _CACHE[reps]


def _shard_inputs(inputs):
    states = np.asarray(inputs["states"], dtype=np.float32)
    actions = np.asarray(inputs["actions"], dtype=np.float32)
    x = np.concatenate([states, actions], axis=-1).reshape(B * N, D)
    shared = {k: np.ascontiguousarray(np.asarray(inputs[k], dtype=np.float32))
              for k in ("Wk", "Wq", "Wv", "W1", "W2")}
    in_maps = []
    for c in range(N_CORES):
        m = {"xin": x[c * M:(c + 1) * M]}
        m.update(shared)
        in_maps.append(m)
    return in_maps


def kernel(**inputs):
    from concourse.bass_utils import run_bass_kernel_spmd
    nc = _get_built(1)
    in_maps = _shard_inputs(inputs)
    res = run_bass_kernel_spmd(nc, in_maps, core_ids=list(range(N_CORES)))
    return _unpack([r["wadv"] for r in res.results])


def _unpack(shards):
    wadv = np.concatenate(shards, axis=0)  # [B*N, 48]
    w = np.ascontiguousarray(wadv[:, 0:32]).reshape(B, N, N, 1)
    adv = np.ascontiguousarray(wadv[:, 32:48]).reshape(B, N, 16)
    return adv, w


if __name__ == "__main__":
    rng = np.random.default_rng(0)
    demo = {
        "states": rng.standard_normal((B, N, OBS), dtype=np.float32),
        "actions": rng.random((B, N, ACT), dtype=np.float32),
        "Wk": (rng.standard_normal((D, D), dtype=np.float32) * 0.05),
        "Wq": (rng.standard_normal((D, D), dtype=np.float32) * 0.05),
        "Wv": (rng.standard_normal((D, D), dtype=np.float32) * 0.05),
        "W1": (rng.standard_normal((64, D), dtype=np.float32) * 0.05),
        "W2": (rng.standard_normal((16, 64), dtype=np.float32) * 0.05),
    }
    adv, w = kernel(**demo)
    print("adv", adv.shape, "w", w.shape)
